# revision 21
# baseline (speedup 1.0000x reference)
"""Cross-attention Trainium2 kernel (self-contained).

Reference computation (B=4, N=M=2048, DIM=1024, H=16, Dh=64):
    q = x @ Wq.T ; k = ctx @ Wk.T ; v = ctx @ Wv.T       (per-head split)
    out = softmax(q k^T / sqrt(Dh)) v                     (per b, h)
    final = out @ Wo.T + bo

Sharding over 8 NeuronCores: core c -> (batch b = c//2, head-group g = c%2).
Each core handles 8 heads (512 of the 1024 inner dims) of one batch and
produces a partial (2048, 1024) output-projection contribution; the host sums
the two partials per batch and adds the bias.

Schedule: linearized step loop s -> (pr, j, i) over 256 attention steps.
Per step: scores(s+2) [PE, 2 row-tiled concurrent K=64 matmuls], exp(s)
[ACT, [128,1024]], attn@V(s-LAG) [PE, 2 col-tiled concurrent M=64 matmuls
into one PSUM bank], plus paced projection work.  The softmax denominator
is accumulated from the exp tiles on DVE/GpSimd (bf16 adds), then a pair of
all-ones-weight matmuls reduces it across partitions AND broadcasts the
result into a PSUM tile in one shot; one reciprocal + one tensor_mul
finishes the normalize (no partition_broadcast, no oc copies).
"""

import numpy as np
import ml_dtypes
from contextlib import ExitStack

import concourse.bass as bass
import concourse.bacc as bacc
import concourse.tile as tile
from concourse import mybir
from concourse import bass_utils

F32 = mybir.dt.float32
BF16 = mybir.dt.bfloat16

B, N, M, DIM = 4, 2048, 2048, 1024
H, DH = 16, 64
NCORES = 8
HG = DIM // 2          # head dims per core (8 heads * 64)
SCALE = DH ** -0.5

NT = N // 512          # q-row tiles of 512 (4)
MT = M // 128          # context-row tiles of 128 (16)
CT = DIM // 128        # contraction tiles for projections (8)
DT = HG // 128         # head-pair tiles per core (4)
NSTEP = DT * NT * MT   # 256 linearized attention steps
LAG = 8                # attn@V emission lag (steps)

_CACHE = {}


def _build_program():
    nc = bacc.Bacc(
        "TRN2",
        target_bir_lowering=False,
        debug=False,
        enable_asserts=False,
        num_devices=NCORES,
    )
    # inputs staged host-side as SBUF images (partition-major); weights are
    # split by first-consumer (pair 0 / rest, couple 0 / 1) so every load is
    # a single contiguous DMA
    xT = nc.dram_tensor("xT", (4, 128, CT, 512), BF16, kind="ExternalInput").ap()
    ctxT = nc.dram_tensor("ctxT", (4, 128, CT, 512), BF16, kind="ExternalInput").ap()
    wq0T = nc.dram_tensor("wq0T", (128, CT, 128), BF16, kind="ExternalInput").ap()
    wqRT = nc.dram_tensor("wqRT", (128, CT, 384), BF16, kind="ExternalInput").ap()
    wk0T = nc.dram_tensor("wk0T", (128, CT, 128), BF16, kind="ExternalInput").ap()
    wkRT = nc.dram_tensor("wkRT", (128, CT, 384), BF16, kind="ExternalInput").ap()
    wv0T = nc.dram_tensor("wv0T", (128, CT, 256), BF16, kind="ExternalInput").ap()
    wv1T = nc.dram_tensor("wv1T", (128, CT, 256), BF16, kind="ExternalInput").ap()
    woT = nc.dram_tensor("woT", (128, DT, DIM), BF16, kind="ExternalInput").ap()
    out = nc.dram_tensor("out", (N, DIM), F32, kind="ExternalOutput").ap()

    with tile.TileContext(nc) as tc:
        _kernel_body(tc, xT, ctxT, wq0T, wqRT, wk0T, wkRT, wv0T, wv1T, woT, out)
    nc.compile()
    return nc


def _kernel_body(tc, xT, ctxT, wq0T, wqRT, wk0T, wkRT, wv0T, wv1T, woT, out):
    nc = tc.nc
    EXP = mybir.ActivationFunctionType.Exp

    with ExitStack() as ctx:
        sb = ctx.enter_context(tc.tile_pool(name="sb", bufs=1))

        xT_sb = sb.tile([128, CT, N], BF16, tag="xT")
        ctxT_sb = sb.tile([128, CT, M], BF16, tag="ctxT")
        wq0_sb = sb.tile([128, CT, 128], BF16, tag="wq0")
        wqR_sb = sb.tile([128, CT, 384], BF16, tag="wqR")
        wk0_sb = sb.tile([128, CT, 128], BF16, tag="wk0")
        wkR_sb = sb.tile([128, CT, 384], BF16, tag="wkR")
        wv0_sb = sb.tile([128, CT, 256], BF16, tag="wv0")
        wv1_sb = sb.tile([128, CT, 256], BF16, tag="wv1")
        wo_sb = sb.tile([128, DT, DIM], BF16, tag="wo")
        qT_sb = sb.tile([128, DT, N], BF16, tag="qT")
        kT_sb = sb.tile([128, DT, M], BF16, tag="kT")
        v_sb = sb.tile([128, MT, 512], BF16, tag="v")
        on_sb = sb.tile([128, DT, N], BF16, tag="on")
        ones_sb = sb.tile([128, 64], BF16, tag="ones")

        sbn = ctx.enter_context(tc.tile_pool(name="sbn", bufs=2))

        # warm the ACT exp table while DMAs stream (saves ~2.7us later)
        dumin = sbn.tile([1, 8], F32, tag="dumin", name="dumin")
        nc.vector.memset(dumin, 0.0)
        dumout = sbn.tile([1, 8], F32, tag="dumout", name="dumout")
        nc.scalar.activation(dumout, dumin, EXP, scale=1.0)

        nc.vector.memset(ones_sb, 1.0)

        # ---- DMA emission: split across both hwdge queues (sync + scalar)
        # so x0 and ctx0 stream in parallel: first exp fires ~7us earlier and
        # the ctx chunks (which gate the early score m-tiles) land sooner.
        def chunk(dst, src, ch):
            return dict(out=dst[:, :, ch * 512:(ch + 1) * 512], in_=src[ch])

        # sync queue: x-path; vector queue: ctx-path (parallel head).  ctx
        # chunk 0 is split 0:256 / 256:512 so the first two score m-tiles
        # depend only on the small leading transfer.
        nc.gpsimd.dma_start(out=wk0_sb, in_=wk0T)
        nc.sync.dma_start(out=wq0_sb, in_=wq0T)
        nc.sync.dma_start(**chunk(xT_sb, xT, 0))
        nc.gpsimd.dma_start(out=ctxT_sb[:, :, 0:256], in_=ctxT[0][:, :, 0:256])
        nc.gpsimd.dma_start(out=ctxT_sb[:, :, 256:512],
                            in_=ctxT[0][:, :, 256:512])
        nc.gpsimd.dma_start(out=wv0_sb, in_=wv0T)
        nc.gpsimd.dma_start(**chunk(ctxT_sb, ctxT, 2))
        nc.gpsimd.dma_start(**chunk(ctxT_sb, ctxT, 3))
        nc.sync.dma_start(**chunk(ctxT_sb, ctxT, 1))
        nc.sync.dma_start(**chunk(xT_sb, xT, 1))
        nc.sync.dma_start(**chunk(xT_sb, xT, 2))
        nc.sync.dma_start(**chunk(xT_sb, xT, 3))
        nc.sync.dma_start(out=wqR_sb, in_=wqRT)
        nc.sync.dma_start(out=wkR_sb, in_=wkRT)
        nc.sync.dma_start(out=wv1_sb, in_=wv1T)
        nc.sync.dma_start(out=wo_sb, in_=woT)

        # ---- pools ----
        psp = ctx.enter_context(tc.tile_pool(name="psp", bufs=2, space="PSUM"))
        pss = ctx.enter_context(tc.tile_pool(name="pss", bufs=2, space="PSUM"))
        pso = ctx.enter_context(tc.tile_pool(name="pso", bufs=2, space="PSUM"))
        sba = ctx.enter_context(tc.tile_pool(name="sba", bufs=LAG + 3))
        sbo = ctx.enter_context(tc.tile_pool(name="sbo", bufs=2))
        sbd = ctx.enter_context(tc.tile_pool(name="sbd", bufs=2))
        sbr = ctx.enter_context(tc.tile_pool(name="sbr", bufs=2))

        # warm the PE clock (HAM) with garbage matmuls while DMAs stream;
        # chained onto the proj PSUM ring ahead of the first projection.
        wu = sb.tile([128, 384], BF16, tag="wu")
        nc.vector.memset(wu, 0.25)
        wups = psp.tile([128, 256], F32, tag="proj", name="warm")
        for _ in range(48):
            nc.tensor.matmul(wups, wu[:, 0:128], wu[:, 128:384],
                             start=True, stop=True)

        # ---- paced work generators (PE quanta) ----
        def q_group(pr, jn):
            w = wq0_sb if pr == 0 else wqR_sb
            lo = 0 if pr == 0 else (pr - 1) * 128
            ps = psp.tile([128, 512], F32, tag="proj", name="qg")
            for c in range(CT):
                nc.tensor.matmul(
                    ps,
                    w[:, c, lo:lo + 128],
                    xT_sb[:, c, jn * 512:(jn + 1) * 512],
                    start=(c == 0), stop=(c == CT - 1),
                )
                if c % 2 == 1:
                    yield
            nc.vector.tensor_copy(qT_sb[:, pr, jn * 512:(jn + 1) * 512], ps)

        def k_group(pr, jm):
            w = wk0_sb if pr == 0 else wkR_sb
            lo = 0 if pr == 0 else (pr - 1) * 128
            ps = psp.tile([128, 512], F32, tag="proj", name="kg")
            for c in range(CT):
                nc.tensor.matmul(
                    ps,
                    w[:, c, lo:lo + 128],
                    ctxT_sb[:, c, jm * 512:(jm + 1) * 512],
                    start=(c == 0), stop=(c == CT - 1),
                )
                if c % 2 == 1:
                    yield
            nc.vector.tensor_copy(kT_sb[:, pr, jm * 512:(jm + 1) * 512], ps)

        def v_group(cp, ii):
            # one couple = 4 heads (256 v-dims) x two m-tiles (2ii, 2ii+1):
            # doubling the m-tiles per group halves the DVE cast count
            w = wv0_sb if cp == 0 else wv1_sb
            ps = psp.tile([128, 512], F32, tag="proj", name="vg")
            # the two halves share partitions in one PSUM bank: start=True
            # clears has_written bank-wide for those partitions, so the
            # accumulation groups must run sequentially, never interleaved
            for t in range(2):
                i = 2 * ii + t
                for c in range(CT):
                    nc.tensor.matmul(
                        ps[:, t * 256:(t + 1) * 256],
                        ctxT_sb[:, c, i * 128:(i + 1) * 128],
                        w[:, c, :],
                        start=(c == 0), stop=(c == CT - 1),
                    )
                    if c % 2 == 1:
                        yield
            nc.vector.tensor_copy(
                v_sb[:, 2 * ii:2 * ii + 2, cp * 256:(cp + 1) * 256],
                ps.rearrange("p (i x) -> p i x", x=256),
            )

        def k_part(lo_col, n_col):
            # pair-0 kT sub-block of jm=0 (the head-critical m columns)
            ps = psp.tile([128, 512], F32, tag="proj", name="kp")
            for c in range(CT):
                nc.tensor.matmul(
                    ps[:, 0:n_col],
                    wk0_sb[:, c, 0:128],
                    ctxT_sb[:, c, lo_col:lo_col + n_col],
                    start=(c == 0), stop=(c == CT - 1),
                )
                if c % 2 == 1:
                    yield
            nc.vector.tensor_copy(kT_sb[:, 0, lo_col:lo_col + n_col],
                                  ps[:, 0:n_col])

        def final_group(n128, e, eng=None):
            ps = psp.tile([128, 512], F32, tag="proj", name="fg")
            for t in range(DT):
                nc.tensor.matmul(
                    ps,
                    on_sb[:, t, n128 * 128:(n128 + 1) * 128],
                    wo_sb[:, t, e * 512:(e + 1) * 512],
                    start=(t == 0), stop=(t == DT - 1),
                )
                if t % 2 == 1:
                    yield
            of = sbo.tile([128, 512], F32, tag="of", name="of")
            nc.vector.tensor_copy(of, ps)
            (eng or nc.sync).dma_start(
                out=out[n128 * 128:(n128 + 1) * 128, e * 512:(e + 1) * 512],
                in_=of,
            )

        # denominator chain state, keyed by (pr, j)
        den_cur = {}       # (pr, j) -> dD
        rec_tiles = {}     # (pr, j) -> rec AP for the normalize mul

        def den_group(pr, j):
            # reduce+broadcast the partial sums across partitions via
            # all-ones weights, then reciprocal.
            dD = den_cur.pop((pr, j))
            dps = psp.tile([128, 512], F32, tag="proj", name="dg")
            nc.tensor.matmul(dps[0:64, :], ones_sb, dD[:, 0:512],
                             start=True, stop=True)
            nc.tensor.matmul(dps[64:128, :], ones_sb, dD[:, 512:1024],
                             start=True, stop=True)
            yield
            rec = sbr.tile([128, 512], F32, tag="rec", name="rec")
            nc.vector.reciprocal_approx_fast(out=rec, in_=dps)
            rec_tiles[(pr, j)] = rec

        # ---- deadline-ordered pacer ----
        class Pacer:
            # At most one group is ever mid-emission (self.cur); a suspended
            # group is always finished before any other group starts, so the
            # 2-buffer proj-PSUM ring never wraps onto a live accumulation.
            def __init__(self):
                self.items = []   # list of [deadline, avail, gen]
                self.cur = None
                self._cur_dl = 10 ** 9

            def add(self, dl, avail, gen):
                self.items.append([dl, avail, gen])

            def sort(self):
                self.items.sort(key=lambda it: it[0])

            def run_due(self, s):
                due = any(it[0] <= s for it in self.items) or (
                    self.cur is not None and self._cur_dl <= s)
                if due and self.cur is not None:
                    for _ in self.cur:
                        pass
                    self.cur = None
                while True:
                    hit = None
                    for idx, it in enumerate(self.items):
                        if it[0] <= s:
                            hit = idx
                            break
                    if hit is None:
                        break
                    for _ in self.items.pop(hit)[2]:
                        pass

            def step(self, s, budget):
                for _ in range(budget):
                    while True:
                        if self.cur is None:
                            nxt = None
                            for idx, it in enumerate(self.items):
                                if it[1] <= s:
                                    nxt = self.items.pop(idx)
                                    break
                            if nxt is None:
                                return
                            self._cur_dl = nxt[0]
                            self.cur = nxt[2]
                        try:
                            next(self.cur)
                            break
                        except StopIteration:
                            self.cur = None

            def drain(self):
                if self.cur is not None:
                    for _ in self.cur:
                        pass
                    self.cur = None
                for it in self.items:
                    for _ in it[2]:
                        pass
                self.items = []

        pacer = Pacer()

        def sp(pr, j, i):
            return pr * 64 + j * 16 + i

        # pair-0 projections beyond the upfront batch (avail = conservative
        # DMA-data-arrival step per the dual-queue arrival model)
        pacer.add(2, 2, k_group(0, 1))
        pacer.add(6, 6, k_group(0, 2))
        pacer.add(10, 10, k_group(0, 3))
        v0_avail = [2, 2, 4, 4, 7, 7, 13, 13]
        for ii in range(MT // 2):
            pacer.add(2 * ii + LAG, min(v0_avail[ii], 2 * ii + LAG),
                      v_group(0, ii))
        pacer.add(14, 10, q_group(0, 1))
        pacer.add(30, 16, q_group(0, 2))
        pacer.add(46, 23, q_group(0, 3))
        # later pairs
        for pr in range(1, DT):
            for jn in range(NT):
                pacer.add(sp(pr, jn, 0) - 2, 25, q_group(pr, jn))
            for jm in range(4):
                pacer.add(sp(pr, 0, 4 * jm) - 2, 28, k_group(pr, jm))
        for ii in range(MT // 2):
            pacer.add(sp(2, 0, 2 * ii) + LAG, 30, v_group(1, ii))
        # denominator reduce groups: run right after the block's last exp
        for pr in range(DT):
            for j in range(NT):
                se = sp(pr, j, 15)
                if se >= NSTEP - 1:
                    continue  # final block handled manually before tail AVs
                pacer.add(se + 4, se + 1, den_group(pr, j))
        pacer.sort()
        # output projection for row block jj: available once normalize(3,jj)
        # has been emitted; deadline opportunistic (drained at end).
        # last row block's DMAs ride the scalar queue: the exp stream is
        # done by then and the sync queue is busy flushing earlier blocks.
        for jj in range(NT):
            av = 192 + jj * 16 + 15 + LAG + 1
            for n128 in range(jj * 4, jj * 4 + 4):
                for e in range(2):
                    eng = (nc.scalar if (n128 + e) % 2 else None) \
                        if jj == 3 else None
                    pacer.add(10 ** 6, av, final_group(n128, e, eng))

        # ---- upfront batch (hidden under the initial DMA wait), ordered by
        # DMA arrival: k-mini (ctx cols 0:256) -> q00 (x chunk 0) -> k-rest
        for g in [k_part(0, 256), q_group(0, 0), k_part(256, 256)]:
            for _ in g:
                pass

        # ---- attention step machinery ----
        def scores(pr, j, i):
            # high scheduler priority: exp(s)'s wait is a PE completion-count
            # threshold at scores(s)'s pc position, so scores must not sit
            # behind same-window paced work in the scheduled order.
            s = pss.tile([128, 1024], F32, tag="sc", name="sc")
            with tc.high_priority(offset=64):
                for half in range(2):
                    lo, hi = half * 64, half * 64 + 64
                    nc.tensor.matmul(
                        s[:, half * 512:(half + 1) * 512],
                        kT_sb[lo:hi, pr, i * 128:(i + 1) * 128],
                        qT_sb[lo:hi, pr, j * 512:(j + 1) * 512],
                        start=True, stop=True,
                    )
            return s

        def do_exp(s_tile):
            a = sba.tile([128, 1024], BF16, tag="attn", name="attn")
            nc.scalar.activation(a, s_tile, EXP, scale=SCALE)
            return a

        def den_add(s, a):
            # all on DVE: GpSimd shares the DVE SBUF port and throttles it
            pr, j, i = s // 64, (s // 16) % 4, s % 16
            if i == 0:
                dD = sbd.tile([128, 1024], BF16, tag="dD", name="dD")
                den_cur[(pr, j)] = dD
                nc.vector.tensor_copy(dD, a)
            else:
                dD = den_cur[(pr, j)]
                nc.vector.tensor_add(dD, dD, a)

        # ---- linearized main loop; AV lags the exp stream by LAG steps ----
        fifo = [scores(0, 0, 0), scores(0, 0, 1)]
        afifo = []
        oo = None

        def av_step(sa):
            nonlocal oo
            pr, j, i = sa // 64, (sa // 16) % 4, sa % 16
            if i == 0:
                oo = pso.tile([128, 512], F32, tag="oacc", name="oacc")
            a = afifo.pop(0)
            for half in range(2):
                h = 2 * pr + half
                nc.tensor.matmul(
                    oo[64 * half:64 * half + 64, :],
                    v_sb[:, i, h * 64:(h + 1) * 64],
                    a[:, half * 512:(half + 1) * 512],
                    start=(i == 0), stop=(i == MT - 1),
                )
            if i == MT - 1:
                rec = rec_tiles.pop((pr, j))
                nc.vector.tensor_mul(
                    on_sb[:, pr, j * 512:(j + 1) * 512], oo, rec)

        for s in range(NSTEP):
            pacer.run_due(s)
            if s + 2 < NSTEP:
                s2 = s + 2
                fifo.append(scores(s2 // 64, (s2 // 16) % 4, s2 % 16))
            a = do_exp(fifo.pop(0))
            den_add(s, a)
            afifo.append(a)
            if s >= LAG:
                av_step(s - LAG)
            if s >= 212:
                pacer.step(s, 3)
            else:
                pacer.step(s, 2)
        # final block's den reduce (its avail lies past the last step)
        for _ in den_group(DT - 1, NT - 1):
            pass
        for sa in range(NSTEP - LAG, NSTEP):
            av_step(sa)
        # keep the PE clock warm while the last normalize chain runs on
        # DVE; otherwise HAM re-throttles and the drain-phase output
        # projections execute at half clock.
        wufill = psp.tile([128, 256], F32, tag="proj", name="tailwarm")
        for _ in range(20):
            nc.tensor.matmul(wufill, wu[:, 0:128], wu[:, 128:384],
                             start=True, stop=True)
        pacer.drain()


def kernel(x, context, Wq, Wk, Wv, Wo, bo):
    x = np.asarray(x, dtype=np.float32)
    context = np.asarray(context, dtype=np.float32)
    Wq = np.asarray(Wq, dtype=np.float32)
    Wk = np.asarray(Wk, dtype=np.float32)
    Wv = np.asarray(Wv, dtype=np.float32)
    Wo = np.asarray(Wo, dtype=np.float32)
    bo = np.asarray(bo, dtype=np.float32)

    if "nc" not in _CACHE:
        _CACHE["nc"] = _build_program()
    nc = _CACHE["nc"]

    in_maps = _make_in_maps(x, context, Wq, Wk, Wv, Wo)
    res = bass_utils.run_bass_kernel_spmd(nc, in_maps, core_ids=list(range(NCORES)))

    final = np.empty((B, N, DIM), dtype=np.float32)
    for b in range(B):
        final[b] = res.results[2 * b]["out"] + res.results[2 * b + 1]["out"] + bo
    return final


def _img_w(a):
    # DRAM->SBUF weight image: (K=1024, F) -> (128, K//128, F), p-major
    return np.ascontiguousarray(
        a.reshape(-1, 128, a.shape[1]).transpose(1, 0, 2))


def _img_x(a):
    # activation image, column-chunked: (1024, 2048) -> (4, 128, 8, 512)
    return np.ascontiguousarray(
        a.reshape(CT, 128, 4, 512).transpose(2, 1, 0, 3))


def _make_in_maps(x, context, Wq, Wk, Wv, Wo):
    bf = ml_dtypes.bfloat16
    xT = [_img_x(np.ascontiguousarray(x[b].T).astype(bf)) for b in range(B)]
    ctxT = [_img_x(np.ascontiguousarray(context[b].T).astype(bf)) for b in range(B)]
    wT = {}
    for g in range(2):
        sl = slice(g * HG, (g + 1) * HG)
        wq = _img_w(np.ascontiguousarray(Wq[sl, :].T).astype(bf))
        wk = _img_w(np.ascontiguousarray(Wk[sl, :].T).astype(bf))
        wv = _img_w(np.ascontiguousarray(Wv[sl, :].T).astype(bf))
        wT[g] = {
            "wq0T": np.ascontiguousarray(wq[:, :, 0:128]),
            "wqRT": np.ascontiguousarray(wq[:, :, 128:512]),
            "wk0T": np.ascontiguousarray(wk[:, :, 0:128]),
            "wkRT": np.ascontiguousarray(wk[:, :, 128:512]),
            "wv0T": np.ascontiguousarray(wv[:, :, 0:256]),
            "wv1T": np.ascontiguousarray(wv[:, :, 256:512]),
            "woT": _img_w(np.ascontiguousarray(Wo[:, sl].T).astype(bf)),
        }
    in_maps = []
    for c in range(NCORES):
        b, g = c // 2, c % 2
        m = {"xT": xT[b], "ctxT": ctxT[b]}
        m.update(wT[g])
        in_maps.append(m)
    return in_maps


def timed_run(inp, trace_dir=None):
    """Run with NTFF tracing; returns HW exec time in ns (or None)."""
    if "nc" not in _CACHE:
        _CACHE["nc"] = _build_program()
    nc = _CACHE["nc"]
    in_maps = _make_in_maps(
        np.asarray(inp["x"], np.float32), np.asarray(inp["context"], np.float32),
        np.asarray(inp["Wq"], np.float32), np.asarray(inp["Wk"], np.float32),
        np.asarray(inp["Wv"], np.float32), np.asarray(inp["Wo"], np.float32))
    res = bass_utils.run_bass_kernel_spmd(
        nc, in_maps, core_ids=list(range(NCORES)), trace=True, tmpdir=trace_dir)
    return res.exec_time_ns


# revision 22
# speedup vs baseline: 1.1902x; 1.1902x over previous
"""Cross-attention Trainium2 kernel (self-contained).

Reference computation (B=4, N=M=2048, DIM=1024, H=16, Dh=64):
    q = x @ Wq.T ; k = ctx @ Wk.T ; v = ctx @ Wv.T       (per-head split)
    out = softmax(q k^T / sqrt(Dh)) v                     (per b, h)
    final = out @ Wo.T + bo

Sharding over 8 NeuronCores: core c -> (batch b = c//2, head-group g = c%2).
Each core handles 8 heads (512 of the 1024 inner dims) of one batch and
produces a partial (2048, 1024) output-projection contribution; the host sums
the two partials per batch and adds the bias.

Schedule: linearized step loop s -> (pr, j, i) over 256 attention steps.
Per step: scores(s+2) [PE, 2 row-tiled concurrent K=64 matmuls], exp(s)
[ACT, [128,1024]], attn@V(s-LAG) [PE, 2 col-tiled concurrent M=64 matmuls
into one PSUM bank], plus paced projection work.  The softmax denominator
is accumulated from the exp tiles on DVE/GpSimd (bf16 adds), then a pair of
all-ones-weight matmuls reduces it across partitions AND broadcasts the
result into a PSUM tile in one shot; one reciprocal + one tensor_mul
finishes the normalize (no partition_broadcast, no oc copies).
"""

import numpy as np
import ml_dtypes
from contextlib import ExitStack

import concourse.bass as bass
import concourse.bacc as bacc
import concourse.tile as tile
from concourse import mybir
from concourse import bass_utils

F32 = mybir.dt.float32
BF16 = mybir.dt.bfloat16

B, N, M, DIM = 4, 2048, 2048, 1024
H, DH = 16, 64
NCORES = 8
HG = DIM // 2          # head dims per core (8 heads * 64)
SCALE = DH ** -0.5

NT = N // 512          # q-row tiles of 512 (4)
MT = M // 128          # context-row tiles of 128 (16)
CT = DIM // 128        # contraction tiles for projections (8)
DT = HG // 128         # head-pair tiles per core (4)
NSTEP = DT * NT * MT   # 256 linearized attention steps
LAG = 8                # attn@V emission lag (steps)

_CACHE = {}


def _build_program():
    nc = bacc.Bacc(
        "TRN2",
        target_bir_lowering=False,
        debug=False,
        enable_asserts=False,
        num_devices=NCORES,
    )
    # inputs staged host-side as SBUF images (partition-major); weights are
    # split by first-consumer (pair 0 / rest, couple 0 / 1) so every load is
    # a single contiguous DMA
    xT = nc.dram_tensor("xT", (4, 128, CT, 512), BF16, kind="ExternalInput").ap()
    ctxT = nc.dram_tensor("ctxT", (4, 128, CT, 512), BF16, kind="ExternalInput").ap()
    wq0T = nc.dram_tensor("wq0T", (128, CT, 128), BF16, kind="ExternalInput").ap()
    wqRT = nc.dram_tensor("wqRT", (128, CT, 384), BF16, kind="ExternalInput").ap()
    wk0T = nc.dram_tensor("wk0T", (128, CT, 128), BF16, kind="ExternalInput").ap()
    wkRT = nc.dram_tensor("wkRT", (128, CT, 384), BF16, kind="ExternalInput").ap()
    wv0T = nc.dram_tensor("wv0T", (128, CT, 256), BF16, kind="ExternalInput").ap()
    wv1T = nc.dram_tensor("wv1T", (128, CT, 256), BF16, kind="ExternalInput").ap()
    woT = nc.dram_tensor("woT", (128, DT, DIM), BF16, kind="ExternalInput").ap()
    out = nc.dram_tensor("out", (N, DIM), F32, kind="ExternalOutput").ap()

    with tile.TileContext(nc) as tc:
        _kernel_body(tc, xT, ctxT, wq0T, wqRT, wk0T, wkRT, wv0T, wv1T, woT, out)
    nc.compile()
    return nc


def _kernel_body(tc, xT, ctxT, wq0T, wqRT, wk0T, wkRT, wv0T, wv1T, woT, out):
    nc = tc.nc
    EXP = mybir.ActivationFunctionType.Exp

    with ExitStack() as ctx:
        sb = ctx.enter_context(tc.tile_pool(name="sb", bufs=1))

        xT_sb = sb.tile([128, CT, N], BF16, tag="xT")
        ctxT_sb = sb.tile([128, CT, M], BF16, tag="ctxT")
        wq0_sb = sb.tile([128, CT, 128], BF16, tag="wq0")
        wqR_sb = sb.tile([128, CT, 384], BF16, tag="wqR")
        wk0_sb = sb.tile([128, CT, 128], BF16, tag="wk0")
        wkR_sb = sb.tile([128, CT, 384], BF16, tag="wkR")
        wv0_sb = sb.tile([128, CT, 256], BF16, tag="wv0")
        wv1_sb = sb.tile([128, CT, 256], BF16, tag="wv1")
        wo_sb = sb.tile([128, DT, DIM], BF16, tag="wo")
        qT_sb = sb.tile([128, DT, N], BF16, tag="qT")
        kT_sb = sb.tile([128, DT, M], BF16, tag="kT")
        v_sb = sb.tile([128, MT, 512], BF16, tag="v")
        on_sb = sb.tile([128, DT, N], BF16, tag="on")
        ones_sb = sb.tile([128, 64], BF16, tag="ones")

        sbn = ctx.enter_context(tc.tile_pool(name="sbn", bufs=2))

        # warm the ACT exp table while DMAs stream (saves ~2.7us later)
        dumin = sbn.tile([1, 8], F32, tag="dumin", name="dumin")
        nc.vector.memset(dumin, 0.0)
        dumout = sbn.tile([1, 8], F32, tag="dumout", name="dumout")
        nc.scalar.activation(dumout, dumin, EXP, scale=1.0)

        nc.vector.memset(ones_sb, 1.0)

        # ---- DMA emission: split across both hwdge queues (sync + scalar)
        # so x0 and ctx0 stream in parallel: first exp fires ~7us earlier and
        # the ctx chunks (which gate the early score m-tiles) land sooner.
        def chunk(dst, src, ch):
            return dict(out=dst[:, :, ch * 512:(ch + 1) * 512], in_=src[ch])

        # single sync queue (dual-queue splits measured slower); ctx chunk 0
        # is split 0:256 / 256:512 so the first two score m-tiles depend
        # only on a small leading transfer that lands ~3.5us in.
        nc.sync.dma_start(out=wq0_sb, in_=wq0T)
        nc.sync.dma_start(out=wk0_sb, in_=wk0T)
        nc.sync.dma_start(out=ctxT_sb[:, :, 0:256], in_=ctxT[0][:, :, 0:256])
        nc.sync.dma_start(**chunk(xT_sb, xT, 0))
        nc.sync.dma_start(out=ctxT_sb[:, :, 256:512],
                          in_=ctxT[0][:, :, 256:512])
        nc.sync.dma_start(**chunk(ctxT_sb, ctxT, 1))
        nc.sync.dma_start(out=wv0_sb, in_=wv0T)
        nc.sync.dma_start(**chunk(ctxT_sb, ctxT, 2))
        nc.sync.dma_start(**chunk(ctxT_sb, ctxT, 3))
        nc.sync.dma_start(**chunk(xT_sb, xT, 1))
        nc.sync.dma_start(**chunk(xT_sb, xT, 2))
        nc.sync.dma_start(**chunk(xT_sb, xT, 3))
        nc.sync.dma_start(out=wqR_sb, in_=wqRT)
        nc.sync.dma_start(out=wkR_sb, in_=wkRT)
        nc.sync.dma_start(out=wv1_sb, in_=wv1T)
        nc.sync.dma_start(out=wo_sb, in_=woT)

        # ---- pools ----
        psp = ctx.enter_context(tc.tile_pool(name="psp", bufs=2, space="PSUM"))
        pss = ctx.enter_context(tc.tile_pool(name="pss", bufs=2, space="PSUM"))
        pso = ctx.enter_context(tc.tile_pool(name="pso", bufs=2, space="PSUM"))
        sba = ctx.enter_context(tc.tile_pool(name="sba", bufs=LAG + 3))
        sbo = ctx.enter_context(tc.tile_pool(name="sbo", bufs=2))
        sbd = ctx.enter_context(tc.tile_pool(name="sbd", bufs=2))
        sbr = ctx.enter_context(tc.tile_pool(name="sbr", bufs=2))

        # warm the PE clock (HAM) with garbage matmuls while DMAs stream;
        # chained onto the proj PSUM ring ahead of the first projection.
        wu = sb.tile([128, 384], BF16, tag="wu")
        nc.vector.memset(wu, 0.25)
        wups = psp.tile([128, 256], F32, tag="proj", name="warm")
        for _ in range(48):
            nc.tensor.matmul(wups, wu[:, 0:128], wu[:, 128:384],
                             start=True, stop=True)

        # ---- paced work generators (PE quanta) ----
        def q_group(pr, jn):
            w = wq0_sb if pr == 0 else wqR_sb
            lo = 0 if pr == 0 else (pr - 1) * 128
            ps = psp.tile([128, 512], F32, tag="proj", name="qg")
            for c in range(CT):
                nc.tensor.matmul(
                    ps,
                    w[:, c, lo:lo + 128],
                    xT_sb[:, c, jn * 512:(jn + 1) * 512],
                    start=(c == 0), stop=(c == CT - 1),
                )
                if c % 2 == 1:
                    yield
            nc.vector.tensor_copy(qT_sb[:, pr, jn * 512:(jn + 1) * 512], ps)

        def k_group(pr, jm):
            w = wk0_sb if pr == 0 else wkR_sb
            lo = 0 if pr == 0 else (pr - 1) * 128
            ps = psp.tile([128, 512], F32, tag="proj", name="kg")
            for c in range(CT):
                nc.tensor.matmul(
                    ps,
                    w[:, c, lo:lo + 128],
                    ctxT_sb[:, c, jm * 512:(jm + 1) * 512],
                    start=(c == 0), stop=(c == CT - 1),
                )
                if c % 2 == 1:
                    yield
            nc.vector.tensor_copy(kT_sb[:, pr, jm * 512:(jm + 1) * 512], ps)

        def v_group(cp, ii):
            # one couple = 4 heads (256 v-dims) x two m-tiles (2ii, 2ii+1):
            # doubling the m-tiles per group halves the DVE cast count
            w = wv0_sb if cp == 0 else wv1_sb
            ps = psp.tile([128, 512], F32, tag="proj", name="vg")
            # the two halves share partitions in one PSUM bank: start=True
            # clears has_written bank-wide for those partitions, so the
            # accumulation groups must run sequentially, never interleaved
            for t in range(2):
                i = 2 * ii + t
                for c in range(CT):
                    nc.tensor.matmul(
                        ps[:, t * 256:(t + 1) * 256],
                        ctxT_sb[:, c, i * 128:(i + 1) * 128],
                        w[:, c, :],
                        start=(c == 0), stop=(c == CT - 1),
                    )
                    if c % 2 == 1:
                        yield
            nc.vector.tensor_copy(
                v_sb[:, 2 * ii:2 * ii + 2, cp * 256:(cp + 1) * 256],
                ps.rearrange("p (i x) -> p i x", x=256),
            )

        def k_part(lo_col, n_col):
            # pair-0 kT sub-block of jm=0 (the head-critical m columns)
            ps = psp.tile([128, 512], F32, tag="proj", name="kp")
            for c in range(CT):
                nc.tensor.matmul(
                    ps[:, 0:n_col],
                    wk0_sb[:, c, 0:128],
                    ctxT_sb[:, c, lo_col:lo_col + n_col],
                    start=(c == 0), stop=(c == CT - 1),
                )
                if c % 2 == 1:
                    yield
            nc.vector.tensor_copy(kT_sb[:, 0, lo_col:lo_col + n_col],
                                  ps[:, 0:n_col])

        def final_group(n128, e, eng=None):
            ps = psp.tile([128, 512], F32, tag="proj", name="fg")
            for t in range(DT):
                nc.tensor.matmul(
                    ps,
                    on_sb[:, t, n128 * 128:(n128 + 1) * 128],
                    wo_sb[:, t, e * 512:(e + 1) * 512],
                    start=(t == 0), stop=(t == DT - 1),
                )
                if t % 2 == 1:
                    yield
            of = sbo.tile([128, 512], F32, tag="of", name="of")
            nc.vector.tensor_copy(of, ps)
            (eng or nc.sync).dma_start(
                out=out[n128 * 128:(n128 + 1) * 128, e * 512:(e + 1) * 512],
                in_=of,
            )

        # denominator chain state, keyed by (pr, j)
        den_cur = {}       # (pr, j) -> dD
        rec_tiles = {}     # (pr, j) -> rec AP for the normalize mul

        def den_group(pr, j):
            # reduce+broadcast the partial sums across partitions via
            # all-ones weights, then reciprocal.
            dD = den_cur.pop((pr, j))
            dps = psp.tile([128, 512], F32, tag="proj", name="dg")
            nc.tensor.matmul(dps[0:64, :], ones_sb, dD[:, 0:512],
                             start=True, stop=True)
            nc.tensor.matmul(dps[64:128, :], ones_sb, dD[:, 512:1024],
                             start=True, stop=True)
            yield
            rec = sbr.tile([128, 512], F32, tag="rec", name="rec")
            nc.vector.reciprocal_approx_fast(out=rec, in_=dps)
            rec_tiles[(pr, j)] = rec

        # ---- deadline-ordered pacer ----
        class Pacer:
            # At most one group is ever mid-emission (self.cur); a suspended
            # group is always finished before any other group starts, so the
            # 2-buffer proj-PSUM ring never wraps onto a live accumulation.
            def __init__(self):
                self.items = []   # list of [deadline, avail, gen]
                self.cur = None
                self._cur_dl = 10 ** 9

            def add(self, dl, avail, gen):
                self.items.append([dl, avail, gen])

            def sort(self):
                self.items.sort(key=lambda it: it[0])

            def run_due(self, s):
                due = any(it[0] <= s for it in self.items) or (
                    self.cur is not None and self._cur_dl <= s)
                if due and self.cur is not None:
                    for _ in self.cur:
                        pass
                    self.cur = None
                while True:
                    hit = None
                    for idx, it in enumerate(self.items):
                        if it[0] <= s:
                            hit = idx
                            break
                    if hit is None:
                        break
                    for _ in self.items.pop(hit)[2]:
                        pass

            def step(self, s, budget):
                for _ in range(budget):
                    while True:
                        if self.cur is None:
                            nxt = None
                            for idx, it in enumerate(self.items):
                                if it[1] <= s:
                                    nxt = self.items.pop(idx)
                                    break
                            if nxt is None:
                                return
                            self._cur_dl = nxt[0]
                            self.cur = nxt[2]
                        try:
                            next(self.cur)
                            break
                        except StopIteration:
                            self.cur = None

            def drain(self):
                if self.cur is not None:
                    for _ in self.cur:
                        pass
                    self.cur = None
                for it in self.items:
                    for _ in it[2]:
                        pass
                self.items = []

        pacer = Pacer()

        def sp(pr, j, i):
            return pr * 64 + j * 16 + i

        # pair-0 projections beyond the upfront batch (avail = conservative
        # DMA-data-arrival step per the dual-queue arrival model)
        pacer.add(2, 2, k_group(0, 1))
        pacer.add(6, 6, k_group(0, 2))
        pacer.add(10, 10, k_group(0, 3))
        v0_avail = [4, 4, 7, 7, 10, 10, 10, 10]
        for ii in range(MT // 2):
            pacer.add(2 * ii + LAG, min(v0_avail[ii], 2 * ii + LAG),
                      v_group(0, ii))
        pacer.add(14, 13, q_group(0, 1))
        pacer.add(30, 16, q_group(0, 2))
        pacer.add(46, 19, q_group(0, 3))
        # later pairs
        for pr in range(1, DT):
            for jn in range(NT):
                pacer.add(sp(pr, jn, 0) - 2, 22, q_group(pr, jn))
            for jm in range(4):
                pacer.add(sp(pr, 0, 4 * jm) - 2, 24, k_group(pr, jm))
        for ii in range(MT // 2):
            pacer.add(sp(2, 0, 2 * ii) + LAG, 26, v_group(1, ii))
        # denominator reduce groups: run right after the block's last exp
        for pr in range(DT):
            for j in range(NT):
                se = sp(pr, j, 15)
                if se >= NSTEP - 1:
                    continue  # final block handled manually before tail AVs
                pacer.add(se + 4, se + 1, den_group(pr, j))
        pacer.sort()
        # output projection for row block jj: available once normalize(3,jj)
        # has been emitted; deadline opportunistic (drained at end).
        # last row block's DMAs ride the scalar queue: the exp stream is
        # done by then and the sync queue is busy flushing earlier blocks.
        for jj in range(NT):
            av = 192 + jj * 16 + 15 + LAG + 1
            for n128 in range(jj * 4, jj * 4 + 4):
                for e in range(2):
                    eng = (nc.scalar if (n128 + e) % 2 else None) \
                        if jj == 3 else None
                    pacer.add(10 ** 6, av, final_group(n128, e, eng))

        # ---- upfront batch (hidden under the initial DMA wait), ordered by
        # DMA arrival: k-mini (ctx cols 0:256) -> q00 (x chunk 0) -> k-rest
        for g in [k_part(0, 256), q_group(0, 0), k_part(256, 256)]:
            for _ in g:
                pass

        # ---- attention step machinery ----
        def scores(pr, j, i):
            # high scheduler priority: exp(s)'s wait is a PE completion-count
            # threshold at scores(s)'s pc position, so scores must not sit
            # behind same-window paced work in the scheduled order.
            s = pss.tile([128, 1024], F32, tag="sc", name="sc")
            with tc.high_priority(offset=64):
                for half in range(2):
                    lo, hi = half * 64, half * 64 + 64
                    nc.tensor.matmul(
                        s[:, half * 512:(half + 1) * 512],
                        kT_sb[lo:hi, pr, i * 128:(i + 1) * 128],
                        qT_sb[lo:hi, pr, j * 512:(j + 1) * 512],
                        start=True, stop=True,
                    )
            return s

        def do_exp(s_tile):
            a = sba.tile([128, 1024], BF16, tag="attn", name="attn")
            nc.scalar.activation(a, s_tile, EXP, scale=SCALE)
            return a

        def den_add(s, a):
            # all on DVE: GpSimd shares the DVE SBUF port and throttles it
            pr, j, i = s // 64, (s // 16) % 4, s % 16
            if i == 0:
                dD = sbd.tile([128, 1024], BF16, tag="dD", name="dD")
                den_cur[(pr, j)] = dD
                nc.vector.tensor_copy(dD, a)
            else:
                dD = den_cur[(pr, j)]
                nc.vector.tensor_add(dD, dD, a)

        # ---- linearized main loop; AV lags the exp stream by LAG steps ----
        fifo = [scores(0, 0, 0), scores(0, 0, 1)]
        afifo = []
        oo = None

        def av_step(sa):
            nonlocal oo
            pr, j, i = sa // 64, (sa // 16) % 4, sa % 16
            if i == 0:
                oo = pso.tile([128, 512], F32, tag="oacc", name="oacc")
            a = afifo.pop(0)
            for half in range(2):
                h = 2 * pr + half
                nc.tensor.matmul(
                    oo[64 * half:64 * half + 64, :],
                    v_sb[:, i, h * 64:(h + 1) * 64],
                    a[:, half * 512:(half + 1) * 512],
                    start=(i == 0), stop=(i == MT - 1),
                )
            if i == MT - 1:
                rec = rec_tiles.pop((pr, j))
                nc.vector.tensor_mul(
                    on_sb[:, pr, j * 512:(j + 1) * 512], oo, rec)

        for s in range(NSTEP):
            pacer.run_due(s)
            if s + 2 < NSTEP:
                s2 = s + 2
                fifo.append(scores(s2 // 64, (s2 // 16) % 4, s2 % 16))
            a = do_exp(fifo.pop(0))
            den_add(s, a)
            afifo.append(a)
            if s >= LAG:
                av_step(s - LAG)
            if s >= 212:
                pacer.step(s, 3)
            elif s >= 200:
                pacer.step(s, 3 if s % 2 == 1 else 2)
            elif 8 <= s < 72:
                # catch-up region: the pacer runs a deficit while the input
                # DMA stream gates avails; ACT stalls here give the PE slack
                pacer.step(s, 3 if s % 2 == 1 else 2)
            else:
                pacer.step(s, 3 if s % 4 == 3 else 2)
        # final block's den reduce (its avail lies past the last step)
        for _ in den_group(DT - 1, NT - 1):
            pass
        for sa in range(NSTEP - LAG, NSTEP):
            av_step(sa)
        # keep the PE clock warm while the last normalize chain runs on
        # DVE; otherwise HAM re-throttles and the drain-phase output
        # projections execute at half clock.
        wufill = psp.tile([128, 256], F32, tag="proj", name="tailwarm")
        for _ in range(20):
            nc.tensor.matmul(wufill, wu[:, 0:128], wu[:, 128:384],
                             start=True, stop=True)
        pacer.drain()


def kernel(x, context, Wq, Wk, Wv, Wo, bo):
    x = np.asarray(x, dtype=np.float32)
    context = np.asarray(context, dtype=np.float32)
    Wq = np.asarray(Wq, dtype=np.float32)
    Wk = np.asarray(Wk, dtype=np.float32)
    Wv = np.asarray(Wv, dtype=np.float32)
    Wo = np.asarray(Wo, dtype=np.float32)
    bo = np.asarray(bo, dtype=np.float32)

    if "nc" not in _CACHE:
        _CACHE["nc"] = _build_program()
    nc = _CACHE["nc"]

    in_maps = _make_in_maps(x, context, Wq, Wk, Wv, Wo)
    res = bass_utils.run_bass_kernel_spmd(nc, in_maps, core_ids=list(range(NCORES)))

    final = np.empty((B, N, DIM), dtype=np.float32)
    for b in range(B):
        final[b] = res.results[2 * b]["out"] + res.results[2 * b + 1]["out"] + bo
    return final


def _img_w(a):
    # DRAM->SBUF weight image: (K=1024, F) -> (128, K//128, F), p-major
    return np.ascontiguousarray(
        a.reshape(-1, 128, a.shape[1]).transpose(1, 0, 2))


def _img_x(a):
    # activation image, column-chunked: (1024, 2048) -> (4, 128, 8, 512)
    return np.ascontiguousarray(
        a.reshape(CT, 128, 4, 512).transpose(2, 1, 0, 3))


def _make_in_maps(x, context, Wq, Wk, Wv, Wo):
    bf = ml_dtypes.bfloat16
    xT = [_img_x(np.ascontiguousarray(x[b].T).astype(bf)) for b in range(B)]
    ctxT = [_img_x(np.ascontiguousarray(context[b].T).astype(bf)) for b in range(B)]
    wT = {}
    for g in range(2):
        sl = slice(g * HG, (g + 1) * HG)
        wq = _img_w(np.ascontiguousarray(Wq[sl, :].T).astype(bf))
        wk = _img_w(np.ascontiguousarray(Wk[sl, :].T).astype(bf))
        wv = _img_w(np.ascontiguousarray(Wv[sl, :].T).astype(bf))
        wT[g] = {
            "wq0T": np.ascontiguousarray(wq[:, :, 0:128]),
            "wqRT": np.ascontiguousarray(wq[:, :, 128:512]),
            "wk0T": np.ascontiguousarray(wk[:, :, 0:128]),
            "wkRT": np.ascontiguousarray(wk[:, :, 128:512]),
            "wv0T": np.ascontiguousarray(wv[:, :, 0:256]),
            "wv1T": np.ascontiguousarray(wv[:, :, 256:512]),
            "woT": _img_w(np.ascontiguousarray(Wo[:, sl].T).astype(bf)),
        }
    in_maps = []
    for c in range(NCORES):
        b, g = c // 2, c % 2
        m = {"xT": xT[b], "ctxT": ctxT[b]}
        m.update(wT[g])
        in_maps.append(m)
    return in_maps


def timed_run(inp, trace_dir=None):
    """Run with NTFF tracing; returns HW exec time in ns (or None)."""
    if "nc" not in _CACHE:
        _CACHE["nc"] = _build_program()
    nc = _CACHE["nc"]
    in_maps = _make_in_maps(
        np.asarray(inp["x"], np.float32), np.asarray(inp["context"], np.float32),
        np.asarray(inp["Wq"], np.float32), np.asarray(inp["Wk"], np.float32),
        np.asarray(inp["Wv"], np.float32), np.asarray(inp["Wo"], np.float32))
    res = bass_utils.run_bass_kernel_spmd(
        nc, in_maps, core_ids=list(range(NCORES)), trace=True, tmpdir=trace_dir)
    return res.exec_time_ns


# revision 23
# speedup vs baseline: 1.1974x; 1.0061x over previous
"""Cross-attention Trainium2 kernel (self-contained).

Reference computation (B=4, N=M=2048, DIM=1024, H=16, Dh=64):
    q = x @ Wq.T ; k = ctx @ Wk.T ; v = ctx @ Wv.T       (per-head split)
    out = softmax(q k^T / sqrt(Dh)) v                     (per b, h)
    final = out @ Wo.T + bo

Sharding over 8 NeuronCores: core c -> (batch b = c//2, head-group g = c%2).
Each core handles 8 heads (512 of the 1024 inner dims) of one batch and
produces a partial (2048, 1024) output-projection contribution; the host sums
the two partials per batch and adds the bias.

Schedule: linearized step loop s -> (pr, j, i) over 256 attention steps.
Per step: scores(s+2) [PE, 2 row-tiled concurrent K=64 matmuls], exp(s)
[ACT, [128,1024]], attn@V(s-LAG) [PE, 2 col-tiled concurrent M=64 matmuls
into one PSUM bank], plus paced projection work.  The softmax denominator
is accumulated from the exp tiles on DVE/GpSimd (bf16 adds), then a pair of
all-ones-weight matmuls reduces it across partitions AND broadcasts the
result into a PSUM tile in one shot; one reciprocal + one tensor_mul
finishes the normalize (no partition_broadcast, no oc copies).
"""

import numpy as np
import ml_dtypes
from contextlib import ExitStack

import concourse.bass as bass
import concourse.bacc as bacc
import concourse.tile as tile
from concourse import mybir
from concourse import bass_utils

F32 = mybir.dt.float32
BF16 = mybir.dt.bfloat16

B, N, M, DIM = 4, 2048, 2048, 1024
H, DH = 16, 64
NCORES = 8
HG = DIM // 2          # head dims per core (8 heads * 64)
SCALE = DH ** -0.5

NT = N // 512          # q-row tiles of 512 (4)
MT = M // 128          # context-row tiles of 128 (16)
CT = DIM // 128        # contraction tiles for projections (8)
DT = HG // 128         # head-pair tiles per core (4)
NSTEP = DT * NT * MT   # 256 linearized attention steps
LAG = 8                # attn@V emission lag (steps)

_CACHE = {}


def _build_program():
    nc = bacc.Bacc(
        "TRN2",
        target_bir_lowering=False,
        debug=False,
        enable_asserts=False,
        num_devices=NCORES,
    )
    # inputs staged host-side as SBUF images (partition-major); weights are
    # split by first-consumer (pair 0 / rest, couple 0 / 1) so every load is
    # a single contiguous DMA
    xT = nc.dram_tensor("xT", (4, 128, CT, 512), BF16, kind="ExternalInput").ap()
    ctxT = nc.dram_tensor("ctxT", (4, 128, CT, 512), BF16, kind="ExternalInput").ap()
    wq0T = nc.dram_tensor("wq0T", (128, CT, 128), BF16, kind="ExternalInput").ap()
    wqRT = nc.dram_tensor("wqRT", (128, CT, 384), BF16, kind="ExternalInput").ap()
    wk0T = nc.dram_tensor("wk0T", (128, CT, 128), BF16, kind="ExternalInput").ap()
    wkRT = nc.dram_tensor("wkRT", (128, CT, 384), BF16, kind="ExternalInput").ap()
    wv0T = nc.dram_tensor("wv0T", (128, CT, 256), BF16, kind="ExternalInput").ap()
    wv1T = nc.dram_tensor("wv1T", (128, CT, 256), BF16, kind="ExternalInput").ap()
    woT = nc.dram_tensor("woT", (128, DT, DIM), BF16, kind="ExternalInput").ap()
    out = nc.dram_tensor("out", (N, DIM), F32, kind="ExternalOutput").ap()

    with tile.TileContext(nc) as tc:
        _kernel_body(tc, xT, ctxT, wq0T, wqRT, wk0T, wkRT, wv0T, wv1T, woT, out)
    nc.compile()
    return nc


def _kernel_body(tc, xT, ctxT, wq0T, wqRT, wk0T, wkRT, wv0T, wv1T, woT, out):
    nc = tc.nc
    EXP = mybir.ActivationFunctionType.Exp

    with ExitStack() as ctx:
        sb = ctx.enter_context(tc.tile_pool(name="sb", bufs=1))

        xT_sb = sb.tile([128, CT, N], BF16, tag="xT")
        ctxT_sb = sb.tile([128, CT, M], BF16, tag="ctxT")
        wq0_sb = sb.tile([128, CT, 128], BF16, tag="wq0")
        wqR_sb = sb.tile([128, CT, 384], BF16, tag="wqR")
        wk0_sb = sb.tile([128, CT, 128], BF16, tag="wk0")
        wkR_sb = sb.tile([128, CT, 384], BF16, tag="wkR")
        wv0_sb = sb.tile([128, CT, 256], BF16, tag="wv0")
        wv1_sb = sb.tile([128, CT, 256], BF16, tag="wv1")
        wo_sb = sb.tile([128, DT, DIM], BF16, tag="wo")
        qT_sb = sb.tile([128, DT, N], BF16, tag="qT")
        kT_sb = sb.tile([128, DT, M], BF16, tag="kT")
        v_sb = sb.tile([128, MT, 512], BF16, tag="v")
        on_sb = sb.tile([128, DT, N], BF16, tag="on")
        ones_sb = sb.tile([128, 64], BF16, tag="ones")

        sbn = ctx.enter_context(tc.tile_pool(name="sbn", bufs=2))

        # warm the ACT exp table while DMAs stream (saves ~2.7us later)
        dumin = sbn.tile([1, 8], F32, tag="dumin", name="dumin")
        nc.vector.memset(dumin, 0.0)
        dumout = sbn.tile([1, 8], F32, tag="dumout", name="dumout")
        nc.scalar.activation(dumout, dumin, EXP, scale=1.0)

        nc.vector.memset(ones_sb, 1.0)

        # ---- DMA emission: split across both hwdge queues (sync + scalar)
        # so x0 and ctx0 stream in parallel: first exp fires ~7us earlier and
        # the ctx chunks (which gate the early score m-tiles) land sooner.
        def chunk(dst, src, ch):
            return dict(out=dst[:, :, ch * 512:(ch + 1) * 512], in_=src[ch])

        # single sync queue (dual-queue splits measured slower); ctx chunk 0
        # is split 0:256 / 256:512 so the first two score m-tiles depend
        # only on a small leading transfer that lands ~3.5us in.
        nc.sync.dma_start(out=wq0_sb, in_=wq0T)
        nc.sync.dma_start(out=wk0_sb, in_=wk0T)
        nc.sync.dma_start(out=ctxT_sb[:, :, 0:256], in_=ctxT[0][:, :, 0:256])
        nc.sync.dma_start(**chunk(xT_sb, xT, 0))
        nc.sync.dma_start(out=ctxT_sb[:, :, 256:512],
                          in_=ctxT[0][:, :, 256:512])
        nc.sync.dma_start(**chunk(ctxT_sb, ctxT, 1))
        nc.sync.dma_start(out=wv0_sb, in_=wv0T)
        nc.sync.dma_start(**chunk(ctxT_sb, ctxT, 2))
        nc.sync.dma_start(**chunk(ctxT_sb, ctxT, 3))
        nc.sync.dma_start(**chunk(xT_sb, xT, 1))
        nc.sync.dma_start(**chunk(xT_sb, xT, 2))
        nc.sync.dma_start(**chunk(xT_sb, xT, 3))
        nc.sync.dma_start(out=wqR_sb, in_=wqRT)
        nc.sync.dma_start(out=wkR_sb, in_=wkRT)
        nc.sync.dma_start(out=wv1_sb, in_=wv1T)
        nc.sync.dma_start(out=wo_sb, in_=woT)

        # ---- pools ----
        psp = ctx.enter_context(tc.tile_pool(name="psp", bufs=2, space="PSUM"))
        pss = ctx.enter_context(tc.tile_pool(name="pss", bufs=2, space="PSUM"))
        pso = ctx.enter_context(tc.tile_pool(name="pso", bufs=2, space="PSUM"))
        sba = ctx.enter_context(tc.tile_pool(name="sba", bufs=LAG + 3))
        sbo = ctx.enter_context(tc.tile_pool(name="sbo", bufs=2))
        sbd = ctx.enter_context(tc.tile_pool(name="sbd", bufs=2))
        sbr = ctx.enter_context(tc.tile_pool(name="sbr", bufs=2))

        # warm the PE clock (HAM) with garbage matmuls while DMAs stream;
        # chained onto the proj PSUM ring ahead of the first projection.
        wu = sb.tile([128, 384], BF16, tag="wu")
        nc.vector.memset(wu, 0.25)
        wups = psp.tile([128, 256], F32, tag="proj", name="warm")
        for _ in range(48):
            nc.tensor.matmul(wups, wu[:, 0:128], wu[:, 128:384],
                             start=True, stop=True)

        # ---- paced work generators (PE quanta) ----
        def q_group(pr, jn):
            w = wq0_sb if pr == 0 else wqR_sb
            lo = 0 if pr == 0 else (pr - 1) * 128
            ps = psp.tile([128, 512], F32, tag="proj", name="qg")
            for c in range(CT):
                nc.tensor.matmul(
                    ps,
                    w[:, c, lo:lo + 128],
                    xT_sb[:, c, jn * 512:(jn + 1) * 512],
                    start=(c == 0), stop=(c == CT - 1),
                )
                if c % 2 == 1:
                    yield
            nc.vector.tensor_copy(qT_sb[:, pr, jn * 512:(jn + 1) * 512], ps)

        def k_group(pr, jm):
            w = wk0_sb if pr == 0 else wkR_sb
            lo = 0 if pr == 0 else (pr - 1) * 128
            ps = psp.tile([128, 512], F32, tag="proj", name="kg")
            for c in range(CT):
                nc.tensor.matmul(
                    ps,
                    w[:, c, lo:lo + 128],
                    ctxT_sb[:, c, jm * 512:(jm + 1) * 512],
                    start=(c == 0), stop=(c == CT - 1),
                )
                if c % 2 == 1:
                    yield
            nc.vector.tensor_copy(kT_sb[:, pr, jm * 512:(jm + 1) * 512], ps)

        def v_group(cp, ii):
            # one couple = 4 heads (256 v-dims) x two m-tiles (2ii, 2ii+1):
            # doubling the m-tiles per group halves the DVE cast count
            w = wv0_sb if cp == 0 else wv1_sb
            ps = psp.tile([128, 512], F32, tag="proj", name="vg")
            # the two halves share partitions in one PSUM bank: start=True
            # clears has_written bank-wide for those partitions, so the
            # accumulation groups must run sequentially, never interleaved
            for t in range(2):
                i = 2 * ii + t
                for c in range(CT):
                    nc.tensor.matmul(
                        ps[:, t * 256:(t + 1) * 256],
                        ctxT_sb[:, c, i * 128:(i + 1) * 128],
                        w[:, c, :],
                        start=(c == 0), stop=(c == CT - 1),
                    )
                    if c % 2 == 1:
                        yield
            nc.vector.tensor_copy(
                v_sb[:, 2 * ii:2 * ii + 2, cp * 256:(cp + 1) * 256],
                ps.rearrange("p (i x) -> p i x", x=256),
            )

        def k_part(lo_col, n_col):
            # pair-0 kT sub-block of jm=0 (the head-critical m columns)
            ps = psp.tile([128, 512], F32, tag="proj", name="kp")
            for c in range(CT):
                nc.tensor.matmul(
                    ps[:, 0:n_col],
                    wk0_sb[:, c, 0:128],
                    ctxT_sb[:, c, lo_col:lo_col + n_col],
                    start=(c == 0), stop=(c == CT - 1),
                )
                if c % 2 == 1:
                    yield
            nc.vector.tensor_copy(kT_sb[:, 0, lo_col:lo_col + n_col],
                                  ps[:, 0:n_col])

        def final_group(n128, e, eng=None):
            ps = psp.tile([128, 512], F32, tag="proj", name="fg")
            for t in range(DT):
                nc.tensor.matmul(
                    ps,
                    on_sb[:, t, n128 * 128:(n128 + 1) * 128],
                    wo_sb[:, t, e * 512:(e + 1) * 512],
                    start=(t == 0), stop=(t == DT - 1),
                )
                if t % 2 == 1:
                    yield
            of = sbo.tile([128, 512], F32, tag="of", name="of")
            nc.vector.tensor_copy(of, ps)
            (eng or nc.sync).dma_start(
                out=out[n128 * 128:(n128 + 1) * 128, e * 512:(e + 1) * 512],
                in_=of,
            )

        # denominator chain state, keyed by (pr, j)
        den_cur = {}       # (pr, j) -> dD
        rec_tiles = {}     # (pr, j) -> rec AP for the normalize mul

        def den_group(pr, j):
            # reduce+broadcast the partial sums across partitions via
            # all-ones weights, then reciprocal.
            dD = den_cur.pop((pr, j))
            dps = psp.tile([128, 512], F32, tag="proj", name="dg")
            nc.tensor.matmul(dps[0:64, :], ones_sb, dD[:, 0:512],
                             start=True, stop=True)
            nc.tensor.matmul(dps[64:128, :], ones_sb, dD[:, 512:1024],
                             start=True, stop=True)
            yield
            rec = sbr.tile([128, 512], F32, tag="rec", name="rec")
            nc.vector.reciprocal_approx_fast(out=rec, in_=dps)
            rec_tiles[(pr, j)] = rec

        # ---- deadline-ordered pacer ----
        class Pacer:
            # At most one group is ever mid-emission (self.cur); a suspended
            # group is always finished before any other group starts, so the
            # 2-buffer proj-PSUM ring never wraps onto a live accumulation.
            def __init__(self):
                self.items = []   # list of [deadline, avail, gen]
                self.cur = None
                self._cur_dl = 10 ** 9

            def add(self, dl, avail, gen):
                self.items.append([dl, avail, gen])

            def sort(self):
                self.items.sort(key=lambda it: it[0])

            def run_due(self, s):
                due = any(it[0] <= s for it in self.items) or (
                    self.cur is not None and self._cur_dl <= s)
                if due and self.cur is not None:
                    for _ in self.cur:
                        pass
                    self.cur = None
                while True:
                    hit = None
                    for idx, it in enumerate(self.items):
                        if it[0] <= s:
                            hit = idx
                            break
                    if hit is None:
                        break
                    for _ in self.items.pop(hit)[2]:
                        pass

            def step(self, s, budget):
                for _ in range(budget):
                    while True:
                        if self.cur is None:
                            nxt = None
                            for idx, it in enumerate(self.items):
                                if it[1] <= s:
                                    nxt = self.items.pop(idx)
                                    break
                            if nxt is None:
                                return
                            self._cur_dl = nxt[0]
                            self.cur = nxt[2]
                        try:
                            next(self.cur)
                            break
                        except StopIteration:
                            self.cur = None

            def drain(self):
                if self.cur is not None:
                    for _ in self.cur:
                        pass
                    self.cur = None
                for it in self.items:
                    for _ in it[2]:
                        pass
                self.items = []

        pacer = Pacer()

        def sp(pr, j, i):
            return pr * 64 + j * 16 + i

        # pair-0 projections beyond the upfront batch (avail = conservative
        # DMA-data-arrival step per the dual-queue arrival model)
        pacer.add(2, 2, k_group(0, 1))
        pacer.add(6, 6, k_group(0, 2))
        pacer.add(10, 10, k_group(0, 3))
        v0_avail = [4, 4, 7, 7, 10, 10, 10, 10]
        for ii in range(MT // 2):
            pacer.add(2 * ii + LAG, min(v0_avail[ii], 2 * ii + LAG),
                      v_group(0, ii))
        pacer.add(14, 13, q_group(0, 1))
        pacer.add(30, 16, q_group(0, 2))
        pacer.add(46, 19, q_group(0, 3))
        # later pairs
        for pr in range(1, DT):
            for jn in range(NT):
                pacer.add(sp(pr, jn, 0) - 2, 22, q_group(pr, jn))
            for jm in range(4):
                pacer.add(sp(pr, 0, 4 * jm) - 2, 24, k_group(pr, jm))
        for ii in range(MT // 2):
            pacer.add(sp(2, 0, 2 * ii) + LAG, 26, v_group(1, ii))
        # denominator reduce groups: run right after the block's last exp
        for pr in range(DT):
            for j in range(NT):
                se = sp(pr, j, 15)
                if se >= NSTEP - 1:
                    continue  # final block handled manually before tail AVs
                pacer.add(se + 4, se + 1, den_group(pr, j))
        pacer.sort()
        # output projection for row block jj: available once normalize(3,jj)
        # has been emitted; deadline opportunistic (drained at end).
        # last row block's DMAs ride the scalar queue: the exp stream is
        # done by then and the sync queue is busy flushing earlier blocks.
        for jj in range(NT):
            av = 192 + jj * 16 + 15 + LAG + 1
            for n128 in range(jj * 4, jj * 4 + 4):
                for e in range(2):
                    eng = (nc.scalar if (n128 + e) % 2 else None) \
                        if jj == 3 else None
                    pacer.add(10 ** 6, av, final_group(n128, e, eng))

        # ---- upfront batch (hidden under the initial DMA wait), ordered by
        # DMA arrival: k-mini (ctx cols 0:256) -> q00 (x chunk 0) -> k-rest,
        # then the front-critical paced groups whose DMAs land during the
        # head window (scores hop past any DMA waits via high_priority).
        for g in [k_part(0, 256), q_group(0, 0), k_part(256, 256)]:
            for _ in g:
                pass
        up = [(2, k_group(0, 1)), (8, v_group(0, 0)), (8, v_group(0, 1)),
              (6, k_group(0, 2))]
        for dl, g in up:
            for it in pacer.items:
                if it[0] == dl and it[2].__name__ == g.__name__:
                    pass
        # remove the corresponding pacer entries and run inline
        def _drop(dl, name):
            for idx, it in enumerate(pacer.items):
                if it[0] == dl and it[2].__name__ == name:
                    pacer.items.pop(idx)
                    return True
            return False
        _drop(2, 'k_group'); _drop(6, 'k_group')
        _drop(LAG, 'v_group'); _drop(2 + LAG, 'v_group')
        for _, g in up:
            for _ in g:
                pass

        # ---- attention step machinery ----
        def scores(pr, j, i):
            # high scheduler priority: exp(s)'s wait is a PE completion-count
            # threshold at scores(s)'s pc position, so scores must not sit
            # behind same-window paced work in the scheduled order.
            s = pss.tile([128, 1024], F32, tag="sc", name="sc")
            with tc.high_priority(offset=64):
                for half in range(2):
                    lo, hi = half * 64, half * 64 + 64
                    nc.tensor.matmul(
                        s[:, half * 512:(half + 1) * 512],
                        kT_sb[lo:hi, pr, i * 128:(i + 1) * 128],
                        qT_sb[lo:hi, pr, j * 512:(j + 1) * 512],
                        start=True, stop=True,
                    )
            return s

        def do_exp(s_tile):
            a = sba.tile([128, 1024], BF16, tag="attn", name="attn")
            nc.scalar.activation(a, s_tile, EXP, scale=SCALE)
            return a

        def den_add(s, a):
            # all on DVE: GpSimd shares the DVE SBUF port and throttles it
            pr, j, i = s // 64, (s // 16) % 4, s % 16
            if i == 0:
                dD = sbd.tile([128, 1024], BF16, tag="dD", name="dD")
                den_cur[(pr, j)] = dD
                nc.vector.tensor_copy(dD, a)
            else:
                dD = den_cur[(pr, j)]
                nc.vector.tensor_add(dD, dD, a)

        # ---- linearized main loop; AV lags the exp stream by LAG steps ----
        fifo = [scores(0, 0, 0), scores(0, 0, 1)]
        afifo = []
        oo = None

        def av_step(sa):
            nonlocal oo
            pr, j, i = sa // 64, (sa // 16) % 4, sa % 16
            if i == 0:
                oo = pso.tile([128, 512], F32, tag="oacc", name="oacc")
            a = afifo.pop(0)
            for half in range(2):
                h = 2 * pr + half
                nc.tensor.matmul(
                    oo[64 * half:64 * half + 64, :],
                    v_sb[:, i, h * 64:(h + 1) * 64],
                    a[:, half * 512:(half + 1) * 512],
                    start=(i == 0), stop=(i == MT - 1),
                )
            if i == MT - 1:
                rec = rec_tiles.pop((pr, j))
                nc.vector.tensor_mul(
                    on_sb[:, pr, j * 512:(j + 1) * 512], oo, rec)

        for s in range(NSTEP):
            pacer.run_due(s)
            if s + 2 < NSTEP:
                s2 = s + 2
                fifo.append(scores(s2 // 64, (s2 // 16) % 4, s2 % 16))
            a = do_exp(fifo.pop(0))
            den_add(s, a)
            afifo.append(a)
            if s >= LAG:
                av_step(s - LAG)
            if s >= 212:
                pacer.step(s, 3)
            elif s >= 200:
                pacer.step(s, 3 if s % 2 == 1 else 2)
            elif 8 <= s < 72:
                # catch-up region: the pacer runs a deficit while the input
                # DMA stream gates avails; ACT stalls here give the PE slack
                pacer.step(s, 3 if s % 2 == 1 else 2)
            else:
                pacer.step(s, 3 if s % 4 == 3 else 2)
        # final block's den reduce (its avail lies past the last step)
        for _ in den_group(DT - 1, NT - 1):
            pass
        for sa in range(NSTEP - LAG, NSTEP):
            av_step(sa)
        # keep the PE clock warm while the last normalize chain runs on
        # DVE; otherwise HAM re-throttles and the drain-phase output
        # projections execute at half clock.
        wufill = psp.tile([128, 256], F32, tag="proj", name="tailwarm")
        for _ in range(20):
            nc.tensor.matmul(wufill, wu[:, 0:128], wu[:, 128:384],
                             start=True, stop=True)
        pacer.drain()


def kernel(x, context, Wq, Wk, Wv, Wo, bo):
    x = np.asarray(x, dtype=np.float32)
    context = np.asarray(context, dtype=np.float32)
    Wq = np.asarray(Wq, dtype=np.float32)
    Wk = np.asarray(Wk, dtype=np.float32)
    Wv = np.asarray(Wv, dtype=np.float32)
    Wo = np.asarray(Wo, dtype=np.float32)
    bo = np.asarray(bo, dtype=np.float32)

    if "nc" not in _CACHE:
        _CACHE["nc"] = _build_program()
    nc = _CACHE["nc"]

    in_maps = _make_in_maps(x, context, Wq, Wk, Wv, Wo)
    res = bass_utils.run_bass_kernel_spmd(nc, in_maps, core_ids=list(range(NCORES)))

    final = np.empty((B, N, DIM), dtype=np.float32)
    for b in range(B):
        final[b] = res.results[2 * b]["out"] + res.results[2 * b + 1]["out"] + bo
    return final


def _img_w(a):
    # DRAM->SBUF weight image: (K=1024, F) -> (128, K//128, F), p-major
    return np.ascontiguousarray(
        a.reshape(-1, 128, a.shape[1]).transpose(1, 0, 2))


def _img_x(a):
    # activation image, column-chunked: (1024, 2048) -> (4, 128, 8, 512)
    return np.ascontiguousarray(
        a.reshape(CT, 128, 4, 512).transpose(2, 1, 0, 3))


def _make_in_maps(x, context, Wq, Wk, Wv, Wo):
    bf = ml_dtypes.bfloat16
    xT = [_img_x(np.ascontiguousarray(x[b].T).astype(bf)) for b in range(B)]
    ctxT = [_img_x(np.ascontiguousarray(context[b].T).astype(bf)) for b in range(B)]
    wT = {}
    for g in range(2):
        sl = slice(g * HG, (g + 1) * HG)
        wq = _img_w(np.ascontiguousarray(Wq[sl, :].T).astype(bf))
        wk = _img_w(np.ascontiguousarray(Wk[sl, :].T).astype(bf))
        wv = _img_w(np.ascontiguousarray(Wv[sl, :].T).astype(bf))
        wT[g] = {
            "wq0T": np.ascontiguousarray(wq[:, :, 0:128]),
            "wqRT": np.ascontiguousarray(wq[:, :, 128:512]),
            "wk0T": np.ascontiguousarray(wk[:, :, 0:128]),
            "wkRT": np.ascontiguousarray(wk[:, :, 128:512]),
            "wv0T": np.ascontiguousarray(wv[:, :, 0:256]),
            "wv1T": np.ascontiguousarray(wv[:, :, 256:512]),
            "woT": _img_w(np.ascontiguousarray(Wo[:, sl].T).astype(bf)),
        }
    in_maps = []
    for c in range(NCORES):
        b, g = c // 2, c % 2
        m = {"xT": xT[b], "ctxT": ctxT[b]}
        m.update(wT[g])
        in_maps.append(m)
    return in_maps


def timed_run(inp, trace_dir=None):
    """Run with NTFF tracing; returns HW exec time in ns (or None)."""
    if "nc" not in _CACHE:
        _CACHE["nc"] = _build_program()
    nc = _CACHE["nc"]
    in_maps = _make_in_maps(
        np.asarray(inp["x"], np.float32), np.asarray(inp["context"], np.float32),
        np.asarray(inp["Wq"], np.float32), np.asarray(inp["Wk"], np.float32),
        np.asarray(inp["Wv"], np.float32), np.asarray(inp["Wo"], np.float32))
    res = bass_utils.run_bass_kernel_spmd(
        nc, in_maps, core_ids=list(range(NCORES)), trace=True, tmpdir=trace_dir)
    return res.exec_time_ns
